# revision 3
# baseline (speedup 1.0000x reference)
"""KAN group-spline kernel for Trainium2 (8 NeuronCores, data-parallel over batch).

Math: out = id_gain[c]*x + spline(clamp(a[c]*x+b[c])) + bias[c], where spline is a
uniform cubic B-spline over K=32 bins with per-group coefficient rows alpha[g].

Device strategy (no gather hardware exists at line rate, so the spline is
evaluated in the "telescoped clamp" basis, which needs no floor/frac/indexing):

    v    = 15.5*a_c*x + 15.5*(b_c+1) + 1          (v in segment S means floor(v)=S)
    F(v) = A_c + sum_{S=0}^{32} g_{c,S}(r_S),      r_S = clamp(v-S, 0, 1)
    g_{c,S}(r) = c1*r + c2*r^2 + c3*r^3            (per channel+segment, host-computed)

  The sum telescopes via spline continuity: below segment 0 / above segment 32 the
  value is exactly the clipped-tap constant, so NO clamp of v is needed at all.

Per 128x4096 tile, per segment S:
  ACT   : q_S = Relu(15.5*x + (oc_c - S))          (scale imm, per-partition bias)
  DVE   : w_S = ((c3*r + c2)*r + c1)*r, r=min(q,1) (ONE custom fused op, 3 per-chan
          scalars via C0/C1/C3-spill)
  GPSIMD: acc += w_S                               (runs parallel to DVE)
Partition dim = (batch,channel) row, so all per-channel params are [P,1] scalars.
"""

import os
import numpy as np

B, C, H, W = 16, 192, 128, 128
K, G = 32, 32
NCORES = 8
SEGS = 33
ROWS = (B // NCORES) * C          # 384 rows per core
FREE = H * W                      # 16384
NCOL = int(os.environ.get("KAN_NCOL", "4096"))
COLT = FREE // NCOL
ROWT = ROWS // 128                # 3

# table column layout (free-dim offsets in the per-rowtile SBUF table)
OFF_IG, OFF_BIAS2, OFF_QB, OFF_C1, OFF_C2, OFF_C3 = 0, 1, 2, 2 + SEGS, 2 + 2 * SEGS, 2 + 3 * SEGS
NTAB = 2 + 4 * SEGS

_BMAT = np.array(
    [
        [1 / 6, -3 / 6, 3 / 6, -1 / 6],
        [4 / 6, 0.0, -6 / 6, 3 / 6],
        [1 / 6, 3 / 6, 3 / 6, -3 / 6],
        [0.0, 0.0, 0.0, 1 / 6],
    ],
    dtype=np.float64,
)  # [tap k, power m]


def build_tables(alpha, a, b, id_gain, bias, group_idx):
    """Host-side exact expansion of the spline into per-(channel,segment) cubic
    coefficients in the telescoped-clamp basis. Returns (scale, tab) where
    tab[rowtile, partition, NTAB] covers rows (batch,channel) = rowtile*128+p."""
    g = group_idx.astype(np.int64)
    alpha_pc = alpha.astype(np.float64)[g]                      # (C, K)
    a64, b64 = a.astype(np.float64), b.astype(np.float64)
    assert np.all(a64 == a64[0]), "fast path needs uniform a (ACT scale is imm)"
    scale = 15.5 * a64[0]
    oc = 15.5 * (b64 + 1.0) + 1.0                               # (C,)

    S = np.arange(SEGS)
    taps = np.clip(S[:, None] - 2 + np.arange(4)[None, :], 0, K - 1)  # (SEGS,4)
    A = alpha_pc[:, taps]                                       # (C, SEGS, 4)
    P = np.einsum("csk,km->csm", A, _BMAT)                      # (C, SEGS, 4)
    c1, c2, c3 = P[..., 1], P[..., 2], P[..., 3]
    Ac = P[:, 0, 0]                                             # value at v=0
    bias2 = bias.astype(np.float64) + Ac

    tab = np.zeros((ROWT, 128, NTAB), dtype=np.float64)
    for t in range(ROWT):
        ch = (t * 128 + np.arange(128)) % C
        tab[t, :, OFF_IG] = id_gain.astype(np.float64)[ch]
        tab[t, :, OFF_BIAS2] = bias2[ch]
        tab[t, :, OFF_QB:OFF_QB + SEGS] = oc[ch, None] - S[None, :]
        tab[t, :, OFF_C1:OFF_C1 + SEGS] = c1[ch]
        tab[t, :, OFF_C2:OFF_C2 + SEGS] = c2[ch]
        tab[t, :, OFF_C3:OFF_C3 + SEGS] = c3[ch]
    return np.float32(scale), tab.astype(np.float32)


def host_emulate(x_rows, scale, tab_t):
    """Numpy emulation of the device program for one row-tile (verification)."""
    q = np.maximum(scale * x_rows[:, None, :] + tab_t[:, OFF_QB:OFF_QB + SEGS, None], 0.0)
    r = np.minimum(q, 1.0)
    c1 = tab_t[:, OFF_C1:OFF_C1 + SEGS, None]
    c2 = tab_t[:, OFF_C2:OFF_C2 + SEGS, None]
    c3 = tab_t[:, OFF_C3:OFF_C3 + SEGS, None]
    w = ((c3 * r + c2) * r + c1) * r
    return x_rows * tab_t[:, OFF_IG, None] + tab_t[:, OFF_BIAS2, None] + w.sum(axis=1)


_PROG_CACHE = {}


def _get_custom_op():
    from concourse.dve_spec import Spec, Src0, C0, C1, C3, One, minn, lower, _spill_c3_to_src1
    from concourse import dve_ops
    from concourse.dve_ops import DveOp, OPS
    from concourse.dve_uop import DveOpSpec

    for op in OPS:
        if op.name == "KAN_SEG":
            return op

    r = minn(Src0, One)
    body = _spill_c3_to_src1(((C3 * r + C1) * r + C0) * r)

    def ref(in0, in1, s0, s1, imm2):
        rr = np.minimum(in0.astype(np.float32), 1.0)
        return ((in1 * rr + s1) * rr + s0) * rr

    spec = Spec(body=body, reference=ref)
    # self-consistent sha: compute what the golden check will compute
    shas = {}
    for ver in ("v3", "v4"):
        tmp = DveOpSpec(name="KAN_SEG", opcode=0, uops=lower(spec, ver=ver), rd1_en=True)
        shas[ver] = tmp.sha(ver)
    op = DveOp("KAN_SEG", spec, subdim=False, uops_sha=shas)
    row = dve_ops._CUSTOM_DVE_ROW_BASE + len(OPS)
    assert row < 0x20
    OPS.append(op)
    dve_ops.CUSTOM_DVE_SPECS[op.name] = spec
    dve_ops._SUB_OPCODE_FOR_NAME[op.name] = row
    assert dve_ops.get_dve_sub_opcode("KAN_SEG") == row
    return op


def _build_program(scale):
    key = ("prog", float(scale), NCOL)
    if key in _PROG_CACHE:
        return _PROG_CACHE[key]

    import concourse.bacc as bacc
    import concourse.mybir as mybir
    from concourse.tile import TileContext
    from concourse.alu_op_type import AluOpType

    kan_op = _get_custom_op()

    nc = bacc.Bacc("TRN2", target_bir_lowering=False, debug=False, num_devices=NCORES)
    x_d = nc.dram_tensor("x", [ROWS, FREE], mybir.dt.float32, kind="ExternalInput").ap()
    tab_d = nc.dram_tensor("tab", [ROWT * 128, NTAB], mybir.dt.float32, kind="ExternalInput").ap()
    out_d = nc.dram_tensor("out", [ROWS, FREE], mybir.dt.float32, kind="ExternalOutput").ap()

    relu = mybir.ActivationFunctionType.Relu

    with TileContext(nc) as tc:
        with (
            tc.tile_pool(name="tabp", bufs=ROWT) as tabp,
            tc.tile_pool(name="xp", bufs=2) as xp,
            tc.tile_pool(name="qp", bufs=3) as qp,
            tc.tile_pool(name="wp", bufs=3) as wp,
            tc.tile_pool(name="accp", bufs=3) as accp,
        ):
            tabs = []
            for t in range(ROWT):
                tt = tabp.tile([128, NTAB], mybir.dt.float32, tag="tab")
                nc.sync.dma_start(tt[:], tab_d[t * 128:(t + 1) * 128, :])
                tabs.append(tt)

            for t in range(ROWT):
                tt = tabs[t]
                for j in range(COLT):
                    rs, cs = slice(t * 128, (t + 1) * 128), slice(j * NCOL, (j + 1) * NCOL)
                    xt = xp.tile([128, NCOL], mybir.dt.float32, tag="x")
                    nc.sync.dma_start(xt[:], x_d[rs, cs])
                    acc = accp.tile([128, NCOL], mybir.dt.float32, tag="acc")
                    # acc0 = x*ig + bias2  (DVE tensor_scalar, 2 per-partition scalars)
                    nc.vector.tensor_scalar(
                        out=acc[:], in0=xt[:],
                        scalar1=tt[:, OFF_IG:OFF_IG + 1], scalar2=tt[:, OFF_BIAS2:OFF_BIAS2 + 1],
                        op0=AluOpType.mult, op1=AluOpType.add,
                    )
                    for s in range(SEGS):
                        q = qp.tile([128, NCOL], mybir.dt.float32, tag="q")
                        nc.scalar.activation(
                            q[:], xt[:], relu,
                            bias=tt[:, OFF_QB + s:OFF_QB + s + 1], scale=float(scale),
                        )
                        w = wp.tile([128, NCOL], mybir.dt.float32, tag="w")
                        nc.vector._custom_dve(
                            kan_op, out=w[:], in0=q[:],
                            in1=tt[:, OFF_C3 + s:OFF_C3 + s + 1],
                            s0=tt[:, OFF_C1 + s:OFF_C1 + s + 1],
                            s1=tt[:, OFF_C2 + s:OFF_C2 + s + 1],
                        )
                        nxt = accp.tile([128, NCOL], mybir.dt.float32, tag="acc")
                        nc.gpsimd.tensor_tensor(nxt[:], acc[:], w[:], AluOpType.add)
                        acc = nxt
                    nc.sync.dma_start(out_d[rs, cs], acc[:])

    nc.compile()
    _PROG_CACHE[key] = nc
    return nc


def kernel(**inputs):
    x = np.asarray(inputs["x"], dtype=np.float32)
    scale, tab = build_tables(
        np.asarray(inputs["alpha"]), np.asarray(inputs["a"]), np.asarray(inputs["b"]),
        np.asarray(inputs["id_gain"]), np.asarray(inputs["bias"]),
        np.asarray(inputs["group_idx"]),
    )
    from concourse import bass_utils

    nc = _build_program(scale)
    tab_flat = np.ascontiguousarray(tab.reshape(ROWT * 128, NTAB))
    xs = x.reshape(NCORES, B // NCORES, C, H, W)
    in_maps = [
        {"x": np.ascontiguousarray(xs[i].reshape(ROWS, FREE)), "tab": tab_flat}
        for i in range(NCORES)
    ]
    trace = bool(int(os.environ.get("KAN_TRACE", "0")))
    res = bass_utils.run_bass_kernel_spmd(
        nc, in_maps, list(range(NCORES)), trace=trace,
        tmpdir=os.environ.get("KAN_TMPDIR") or None,
    )
    if trace and res.exec_time_ns is not None:
        print(f"HW exec time: {res.exec_time_ns} ns")
    out = np.stack([res.results[i]["out"] for i in range(NCORES)])
    return np.ascontiguousarray(out.reshape(B, C, H, W).astype(np.float32))


# revision 6
# speedup vs baseline: 3125.8633x; 3125.8633x over previous
"""KAN group-spline kernel for Trainium2 (8 NeuronCores, data-parallel over batch).

Math: out = id_gain[c]*x + spline(clamp(a[c]*x+b[c])) + bias[c], where spline is a
uniform cubic B-spline over K=32 bins with per-group coefficient rows alpha[g].

Device strategy (no gather hardware exists at line rate, so the spline is
evaluated in the "telescoped clamp" basis, which needs no floor/frac/indexing):

    v    = 15.5*a_c*x + 15.5*(b_c+1) + 1          (v in segment S means floor(v)=S)
    F(v) = A_c + sum_{S=0}^{32} g_{c,S}(r_S),      r_S = clamp(v-S, 0, 1)
    g_{c,S}(r) = c1*r + c2*r^2 + c3*r^3            (per channel+segment, host-computed)

  The sum telescopes via spline continuity: below segment 0 / above segment 32 the
  value is exactly the clipped-tap constant, so NO clamp of v is needed at all.

Per 128x4096 tile, per segment S:
  ACT   : q_S = Relu(15.5*x + (oc_c - S))          (scale imm, per-partition bias)
  DVE   : w_S = ((c3*r + c2)*r + c1)*r, r=min(q,1) (ONE custom fused op, 3 per-chan
          scalars via C0/C1/C3-spill)
  GPSIMD: acc += w_S                               (runs parallel to DVE)
Partition dim = (batch,channel) row, so all per-channel params are [P,1] scalars.
"""

import os
import numpy as np

B, C, H, W = 16, 192, 128, 128
K, G = 32, 32
NCORES = 8
SEGS = 33
ROWS = (B // NCORES) * C          # 384 rows per core
FREE = H * W                      # 16384
NCOL = int(os.environ.get("KAN_NCOL", "4096"))
COLT = FREE // NCOL
ROWT = ROWS // 128                # 3

# table column layout (free-dim offsets in the per-rowtile SBUF table)
OFF_IG, OFF_BIAS2, OFF_QB, OFF_C1, OFF_C2, OFF_C3 = 0, 1, 2, 2 + SEGS, 2 + 2 * SEGS, 2 + 3 * SEGS
NTAB = 2 + 4 * SEGS

_BMAT = np.array(
    [
        [1 / 6, -3 / 6, 3 / 6, -1 / 6],
        [4 / 6, 0.0, -6 / 6, 3 / 6],
        [1 / 6, 3 / 6, 3 / 6, -3 / 6],
        [0.0, 0.0, 0.0, 1 / 6],
    ],
    dtype=np.float64,
)  # [tap k, power m]


def build_tables(alpha, a, b, id_gain, bias, group_idx):
    """Host-side exact expansion of the spline into per-(channel,segment) cubic
    coefficients in the telescoped-clamp basis. Returns (scale, tab) where
    tab[rowtile, partition, NTAB] covers rows (batch,channel) = rowtile*128+p."""
    g = group_idx.astype(np.int64)
    alpha_pc = alpha.astype(np.float64)[g]                      # (C, K)
    a64, b64 = a.astype(np.float64), b.astype(np.float64)
    assert np.all(a64 == a64[0]), "fast path needs uniform a (ACT scale is imm)"
    scale = 15.5 * a64[0]
    oc = 15.5 * (b64 + 1.0) + 1.0                               # (C,)

    S = np.arange(SEGS)
    taps = np.clip(S[:, None] - 2 + np.arange(4)[None, :], 0, K - 1)  # (SEGS,4)
    A = alpha_pc[:, taps]                                       # (C, SEGS, 4)
    P = np.einsum("csk,km->csm", A, _BMAT)                      # (C, SEGS, 4)
    c1, c2, c3 = P[..., 1], P[..., 2], P[..., 3]
    Ac = P[:, 0, 0]                                             # value at v=0
    bias2 = bias.astype(np.float64) + Ac

    tab = np.zeros((ROWT, 128, NTAB), dtype=np.float64)
    for t in range(ROWT):
        ch = (t * 128 + np.arange(128)) % C
        tab[t, :, OFF_IG] = id_gain.astype(np.float64)[ch]
        tab[t, :, OFF_BIAS2] = bias2[ch]
        tab[t, :, OFF_QB:OFF_QB + SEGS] = oc[ch, None] - S[None, :]
        tab[t, :, OFF_C1:OFF_C1 + SEGS] = c1[ch]
        tab[t, :, OFF_C2:OFF_C2 + SEGS] = c2[ch]
        tab[t, :, OFF_C3:OFF_C3 + SEGS] = c3[ch]
    return np.float32(scale), tab.astype(np.float32)


def host_emulate(x_rows, scale, tab_t):
    """Numpy emulation of the device program for one row-tile (verification)."""
    q = np.maximum(scale * x_rows[:, None, :] + tab_t[:, OFF_QB:OFF_QB + SEGS, None], 0.0)
    r = np.minimum(q, 1.0)
    c1 = tab_t[:, OFF_C1:OFF_C1 + SEGS, None]
    c2 = tab_t[:, OFF_C2:OFF_C2 + SEGS, None]
    c3 = tab_t[:, OFF_C3:OFF_C3 + SEGS, None]
    w = ((c3 * r + c2) * r + c1) * r
    return x_rows * tab_t[:, OFF_IG, None] + tab_t[:, OFF_BIAS2, None] + w.sum(axis=1)


_PROG_CACHE = {}


def _get_custom_op():
    from concourse.dve_spec import Spec, Src0, C0, C1, C3, One, minn, lower, _spill_c3_to_src1
    from concourse import dve_ops
    from concourse.dve_ops import DveOp, OPS
    from concourse.dve_uop import DveOpSpec

    for op in OPS:
        if op.name == "KAN_SEG":
            return op

    r = minn(Src0, One)
    body = _spill_c3_to_src1(((C3 * r + C1) * r + C0) * r)

    def ref(in0, in1, s0, s1, imm2):
        rr = np.minimum(in0.astype(np.float32), 1.0)
        return ((in1 * rr + s1) * rr + s0) * rr

    spec = Spec(body=body, reference=ref)
    # self-consistent sha: compute what the golden check will compute
    shas = {}
    for ver in ("v3", "v4"):
        tmp = DveOpSpec(name="KAN_SEG", opcode=0, uops=lower(spec, ver=ver), rd1_en=True)
        shas[ver] = tmp.sha(ver)
    op = DveOp("KAN_SEG", spec, subdim=False, uops_sha=shas)
    row = dve_ops._CUSTOM_DVE_ROW_BASE + len(OPS)
    assert row < 0x20
    OPS.append(op)
    dve_ops.CUSTOM_DVE_SPECS[op.name] = spec
    dve_ops._SUB_OPCODE_FOR_NAME[op.name] = row
    assert dve_ops.get_dve_sub_opcode("KAN_SEG") == row
    return op


def _build_program(scale):
    repeat = int(os.environ.get("KAN_REPEAT", "1"))
    key = ("prog", float(scale), NCOL, repeat)
    if key in _PROG_CACHE:
        return _PROG_CACHE[key]

    import concourse.bacc as bacc
    import concourse.mybir as mybir
    from concourse.tile import TileContext
    from concourse.alu_op_type import AluOpType

    kan_op = _get_custom_op()

    nc = bacc.Bacc("TRN2", target_bir_lowering=False, debug=False, num_devices=NCORES)
    x_d = nc.dram_tensor("x", [ROWS, FREE], mybir.dt.float32, kind="ExternalInput").ap()
    tab_d = nc.dram_tensor("tab", [ROWT * 128, NTAB], mybir.dt.float32, kind="ExternalInput").ap()
    out_d = nc.dram_tensor("out", [ROWS, FREE], mybir.dt.float32, kind="ExternalOutput").ap()

    relu = mybir.ActivationFunctionType.Relu

    with TileContext(nc) as tc:
        with (
            tc.tile_pool(name="tabp", bufs=ROWT) as tabp,
            tc.tile_pool(name="xp", bufs=2) as xp,
            tc.tile_pool(name="qp", bufs=3) as qp,
            tc.tile_pool(name="wp", bufs=3) as wp,
            tc.tile_pool(name="accp", bufs=3) as accp,
        ):
            tabs = []
            for t in range(ROWT):
                tt = tabp.tile([128, NTAB], mybir.dt.float32, tag="tab")
                nc.sync.dma_start(tt[:], tab_d[t * 128:(t + 1) * 128, :])
                tabs.append(tt)

            import contextlib
            loop_ctx = tc.For_i(0, repeat, 1) if repeat > 1 else contextlib.nullcontext()
            with loop_ctx:
                _emit_body(nc, tc, tabs, x_d, out_d, xp, qp, wp, accp, kan_op, scale)

    nc.compile()
    _PROG_CACHE[key] = nc
    return nc


def _emit_body(nc, tc, tabs, x_d, out_d, xp, qp, wp, accp, kan_op, scale):
    import concourse.mybir as mybir
    from concourse.alu_op_type import AluOpType

    relu = mybir.ActivationFunctionType.Relu
    if True:
        if True:
            for t in range(ROWT):
                tt = tabs[t]
                for j in range(COLT):
                    rs, cs = slice(t * 128, (t + 1) * 128), slice(j * NCOL, (j + 1) * NCOL)
                    xt = xp.tile([128, NCOL], mybir.dt.float32, tag="x")
                    nc.sync.dma_start(xt[:], x_d[rs, cs])
                    acc = accp.tile([128, NCOL], mybir.dt.float32, tag="acc")
                    # acc0 = x*ig + bias2  (DVE tensor_scalar, 2 per-partition scalars)
                    nc.vector.tensor_scalar(
                        out=acc[:], in0=xt[:],
                        scalar1=tt[:, OFF_IG:OFF_IG + 1], scalar2=tt[:, OFF_BIAS2:OFF_BIAS2 + 1],
                        op0=AluOpType.mult, op1=AluOpType.add,
                    )
                    for s in range(SEGS):
                        q = qp.tile([128, NCOL], mybir.dt.float32, tag="q")
                        nc.scalar.activation(
                            q[:], xt[:], relu,
                            bias=tt[:, OFF_QB + s:OFF_QB + s + 1], scale=float(scale),
                        )
                        w = wp.tile([128, NCOL], mybir.dt.float32, tag="w")
                        nc.vector._custom_dve(
                            kan_op, out=w[:], in0=q[:],
                            in1=tt[:, OFF_C3 + s:OFF_C3 + s + 1],
                            s0=tt[:, OFF_C1 + s:OFF_C1 + s + 1],
                            s1=tt[:, OFF_C2 + s:OFF_C2 + s + 1],
                        )
                        nxt = accp.tile([128, NCOL], mybir.dt.float32, tag="acc")
                        nc.gpsimd.tensor_tensor(nxt[:], acc[:], w[:], AluOpType.add)
                        acc = nxt
                    nc.sync.dma_start(out_d[rs, cs], acc[:])


def kernel(**inputs):
    x = np.asarray(inputs["x"], dtype=np.float32)
    scale, tab = build_tables(
        np.asarray(inputs["alpha"]), np.asarray(inputs["a"]), np.asarray(inputs["b"]),
        np.asarray(inputs["id_gain"]), np.asarray(inputs["bias"]),
        np.asarray(inputs["group_idx"]),
    )
    from concourse import bass_utils

    nc = _build_program(scale)
    tab_flat = np.ascontiguousarray(tab.reshape(ROWT * 128, NTAB))
    xs = x.reshape(NCORES, B // NCORES, C, H, W)
    in_maps = [
        {"x": np.ascontiguousarray(xs[i].reshape(ROWS, FREE)), "tab": tab_flat}
        for i in range(NCORES)
    ]
    trace = bool(int(os.environ.get("KAN_TRACE", "0")))
    res = bass_utils.run_bass_kernel_spmd(
        nc, in_maps, list(range(NCORES)), trace=trace,
        tmpdir=os.environ.get("KAN_TMPDIR") or None,
    )
    if trace and res.exec_time_ns is not None:
        print(f"HW exec time: {res.exec_time_ns} ns")
    out = np.stack([res.results[i]["out"] for i in range(NCORES)])
    return np.ascontiguousarray(out.reshape(B, C, H, W).astype(np.float32))
